# revision 5
# baseline (speedup 1.0000x reference)
"""ChebConv (K=4, sym norm, lambda_max=2) forward on 8 Trainium2 NeuronCores.

Problem (hardcoded): x [50000, 48] f32, edge_index [2, 1.6M] int64,
weight [4, 48, 48] f32, bias [48] f32 -> out [50000, 48] f32.

Strategy
--------
Math: with lambda_max=2 and sym normalization, L_hat's off-diagonal weight for
edge (j->i) is -dis[j]*dis[i] with dis = deg^-1/2 (deg = source out-counts,
self-loops removed). So prop(h) = -dis * segsum_col(dis[row] * h[row]): all
per-edge weights fold into two dense per-node scalings around a plain
gather + segment-sum; no per-edge multiply on device.

Distribution: core c owns target nodes [c*6250, (c+1)*6250). Edges are
partitioned by target core and sorted into 128-wide target blocks. Per
Chebyshev step: gather dis-scaled features (dma_gather, 256B f32 elems) from
an AllGathered DRAM table, segment-sum via one-hot matmuls on the
TensorEngine (race-free, PSUM-accumulated), apply the recurrence on the
local shard, AllGather the rescaled shard for the next step. Features ride
through the matmul as bf16 hi+lo splits (exact to ~1e-6 rel).

deg uses the same one-hot matmul against a ones vector over a
row-partitioned copy of the edge list.

dma_gather indices are int16, so each block's gather stream is split into
[0, 32768) and [32768, ..) windows (second call gathers from a row-offset
view of the table).

Node order inside a shard is "blocked": node n (r = n % 6250) maps to table
row NPAD*(n//6250) + NBLK*(r%128) + r//128, making every dense DMA
partition-contiguous.
"""

import sys

for _p in ("/opt/trn_rl_repo", "/root/.axon_site/_ro/trn_rl_repo"):
    if _p not in sys.path:
        sys.path.insert(0, _p)

import numpy as np
import ml_dtypes

import concourse.bacc as bacc
import concourse.mybir as mybir
import concourse.tile as tile
from concourse import bass_utils

F32 = mybir.dt.float32
BF16 = mybir.dt.bfloat16
I16 = mybir.dt.int16

P = 128


class Cfg:
    def __init__(self, n_nodes, d_in, d_out, k_cheb, n_cores, split,
                 t_lo, t_hi, t_deg):
        self.N = n_nodes
        self.D = d_in
        self.DO = d_out
        self.K = k_cheb
        self.NC = n_cores
        self.SPLIT = split              # int16 gather window size
        self.NSH = n_nodes // n_cores   # nodes per core
        self.NBLK = (self.NSH + P - 1) // P
        self.NPAD = self.NBLK * P
        self.NTAB = self.NC * self.NPAD
        self.DP = 64                    # padded feature width (256B elems)
        self.T_LO = t_lo
        self.T_HI = t_hi
        self.T_ALL = (t_lo + t_hi) if t_lo is not None else None
        self.T_DEG = t_deg

    def key(self):
        return (self.N, self.D, self.DO, self.K, self.NC, self.SPLIT,
                self.T_LO, self.T_HI, self.T_DEG)


# ----------------------------------------------------------------------------
# host-side data prep
# ----------------------------------------------------------------------------

def _wrap_idx16(vals):
    """[n] ints -> [128, n/16] int16 wrapped layout (idx i at
    (16g + i%16, i//16), replicated for the 8 gpsimd cores)."""
    n = len(vals)
    assert n % 16 == 0
    grid = np.asarray(vals, np.int64).reshape(n // 16, 16).T.astype(np.int16)
    return np.ascontiguousarray(np.tile(grid, (8, 1)))


def host_prep(x, edge_index, weight, bias, cfg):
    N, D, DP, NC, NSH, NBLK = cfg.N, cfg.D, cfg.DP, cfg.NC, cfg.NSH, cfg.NBLK

    rows = np.asarray(edge_index[0], np.int64)
    cols = np.asarray(edge_index[1], np.int64)
    keep = rows != cols
    rows, cols = rows[keep], cols[keep]

    # blocked table row of the source node
    rsh = rows % NSH
    gidx = cfg.NPAD * (rows // NSH) + NBLK * (rsh % P) + rsh // P
    lo_mask = gidx < cfg.SPLIT

    csh = cols % NSH
    core_c = cols // NSH
    colblk = csh // P
    colrel = csh % P
    core_r = rows // NSH
    rowblk = rsh // P
    rowrel = rsh % P

    # ---- order edges + measure tile counts ------------------------------
    t_lo = t_hi = t_deg = 0
    per_core_sorted = []
    for c in range(NC):
        sel = np.nonzero(core_c == c)[0]
        order = np.lexsort((~lo_mask[sel], colblk[sel]))
        sel = sel[order]
        bb = colblk[sel]
        ll = lo_mask[sel]
        dsel = np.nonzero(core_r == c)[0]
        per_core_sorted.append((sel, bb, ll, dsel))
        for b in range(NBLK):
            inb = bb == b
            nlo = int(np.count_nonzero(inb & ll))
            nhi = int(np.count_nonzero(inb)) - nlo
            t_lo = max(t_lo, -(-nlo // P))
            t_hi = max(t_hi, -(-nhi // P))
        bc = np.bincount(rowblk[dsel], minlength=NBLK)
        t_deg = max(t_deg, int(-(-int(bc.max()) // P)))

    if cfg.T_LO is None:
        cfg = Cfg(cfg.N, cfg.D, cfg.DO, cfg.K, cfg.NC, cfg.SPLIT,
                  t_lo, t_hi, t_deg)
    else:
        assert (t_lo <= cfg.T_LO and t_hi <= cfg.T_HI
                and t_deg <= cfg.T_DEG), (t_lo, t_hi, t_deg)

    T_LO, T_HI, T_ALL, T_DEG = cfg.T_LO, cfg.T_HI, cfg.T_ALL, cfg.T_DEG

    # ---- per-core arrays -------------------------------------------------
    per_core = []
    for c in range(NC):
        sel, bb, ll, dsel = per_core_sorted[c]
        g_arr = np.zeros((NBLK, T_ALL * P), np.int64)          # pad idx 0
        c_arr = np.full((NBLK, T_ALL * P), -1.0, np.float32)   # pad colrel -1
        for b in range(NBLK):
            inb = np.nonzero(bb == b)[0]
            nlo = int(np.count_nonzero(ll[inb]))
            nb = len(inb)
            lo_ids = sel[inb[:nlo]]
            hi_ids = sel[inb[nlo:]]
            g_arr[b, :nlo] = gidx[lo_ids]
            c_arr[b, :nlo] = colrel[lo_ids]
            g_arr[b, T_LO * P:T_LO * P + nb - nlo] = gidx[hi_ids] - cfg.SPLIT
            c_arr[b, T_LO * P:T_LO * P + nb - nlo] = colrel[hi_ids]
        idx_sb = _wrap_idx16(g_arr.reshape(-1))
        colrel_sb = np.ascontiguousarray(
            c_arr.reshape(NBLK * T_ALL, P).T)

        d_arr = np.full((NBLK, T_DEG * P), -1.0, np.float32)
        dblk = rowblk[dsel]
        drel = rowrel[dsel]
        for b in range(NBLK):
            inb = np.nonzero(dblk == b)[0]
            d_arr[b, :len(inb)] = drel[inb]
        degrel_sb = np.ascontiguousarray(
            d_arr.reshape(NBLK * T_DEG, P).T)

        xs = np.zeros((P, NBLK, DP), np.float32)
        r0 = c * NSH
        nrows = min(NSH, N - r0)
        xi = np.asarray(x[r0:r0 + nrows], np.float32)
        rr = np.arange(nrows)
        xs[rr % P, rr // P, :D] = xi
        x_sb = np.ascontiguousarray(xs.reshape(P, NBLK * DP))

        per_core.append({
            "gidx": idx_sb, "colrel": colrel_sb, "degrel": degrel_sb,
            "x_shard": x_sb,
        })

    W = np.asarray(weight, np.float32)
    wmat = np.ascontiguousarray(
        W.transpose(1, 0, 2).reshape(D, cfg.K * cfg.DO))
    bias_sb = np.ascontiguousarray(
        np.broadcast_to(np.asarray(bias, np.float32), (P, cfg.DO)))
    iota_sb = np.ascontiguousarray(
        np.broadcast_to(np.arange(P, dtype=np.float32), (P, P))
    ).astype(ml_dtypes.bfloat16)
    ident_sb = np.eye(P, dtype=np.float32)
    ones_sb = np.ones((P, 1), ml_dtypes.bfloat16)

    common = {"wmat": wmat, "bias": bias_sb, "iota": iota_sb,
              "ident": ident_sb, "ones": ones_sb}
    return common, per_core, cfg


# ----------------------------------------------------------------------------
# bass program
# ----------------------------------------------------------------------------

def build_program(cfg):
    NBLK, DP, D, DO, K = cfg.NBLK, cfg.DP, cfg.D, cfg.DO, cfg.K
    T_LO, T_HI, T_ALL, T_DEG = cfg.T_LO, cfg.T_HI, cfg.T_ALL, cfg.T_DEG
    NC, NPAD, NTAB = cfg.NC, cfg.NPAD, cfg.NTAB
    COPY = mybir.ActivationFunctionType.Copy

    nc = bacc.Bacc("TRN2", target_bir_lowering=False, debug=False,
                   num_devices=NC)

    t_x = nc.dram_tensor("x_shard", [P, NBLK * DP], F32, kind="ExternalInput")
    t_gidx = nc.dram_tensor("gidx", [P, NBLK * T_ALL * 8], I16,
                            kind="ExternalInput")
    t_colrel = nc.dram_tensor("colrel", [P, NBLK * T_ALL], F32,
                              kind="ExternalInput")
    t_degrel = nc.dram_tensor("degrel", [P, NBLK * T_DEG], F32,
                              kind="ExternalInput")
    t_wmat = nc.dram_tensor("wmat", [D, K * DO], F32, kind="ExternalInput")
    t_bias = nc.dram_tensor("bias", [P, DO], F32, kind="ExternalInput")
    t_iota = nc.dram_tensor("iota", [P, P], BF16, kind="ExternalInput")
    t_ident = nc.dram_tensor("ident", [P, P], F32, kind="ExternalInput")
    t_ones = nc.dram_tensor("ones", [P, 1], BF16, kind="ExternalInput")
    t_out = nc.dram_tensor("out", [P, NBLK * DO], F32, kind="ExternalOutput")

    bounce = [nc.dram_tensor(f"bounce{k}", [NPAD, DP], F32)
              for k in range(K - 1)]
    table = [nc.dram_tensor(f"table{k}", [NTAB, DP], F32, addr_space="Shared")
             for k in range(K - 1)]
    rg = [list(range(NC))]

    with tile.TileContext(nc) as tc:
        with (
            tc.tile_pool(name="const", bufs=1) as cpool,
            tc.tile_pool(name="state", bufs=1) as spool,
            tc.tile_pool(name="tpool", bufs=3) as tpool,
            tc.tile_pool(name="hpool", bufs=2) as hpool,
            tc.tile_pool(name="work", bufs=2) as wpool,
            tc.tile_pool(name="ohp", bufs=2) as ohpool,
            tc.tile_pool(name="ps", bufs=3, space="PSUM") as pspool,
            tc.tile_pool(name="aux", bufs=4, space="PSUM") as auxpool,
        ):
            iota = cpool.tile([P, P], BF16)
            nc.sync.dma_start(iota[:], t_iota[:, :])
            ident = cpool.tile([P, P], F32)
            nc.sync.dma_start(ident[:], t_ident[:, :])
            ones = cpool.tile([P, 1], BF16)
            nc.sync.dma_start(ones[:], t_ones[:, :])
            bias_sb = cpool.tile([P, DO], F32)
            nc.sync.dma_start(bias_sb[:], t_bias[:, :])
            wmat = cpool.tile([D, K * DO], F32)
            nc.sync.dma_start(wmat[:], t_wmat[:, :])
            gidx = cpool.tile([P, NBLK * T_ALL * 8], I16)
            nc.sync.dma_start(gidx[:], t_gidx[:, :])
            colrel = cpool.tile([P, NBLK * T_ALL], F32)
            nc.sync.dma_start(colrel[:], t_colrel[:, :])
            degrel = cpool.tile([P, NBLK * T_DEG], F32)
            nc.sync.dma_start(degrel[:], t_degrel[:, :])
            xs = tpool.tile([P, NBLK, DP], F32, tag="tn")
            nc.sync.dma_start(xs[:], t_x[:, :].rearrange("p (b f) -> p b f",
                                                         b=NBLK))

            # ---- deg pass -------------------------------------------------
            deg = spool.tile([P, NBLK], F32)
            for b in range(NBLK):
                oh = ohpool.tile([P, T_DEG, P], BF16, tag="oh")
                nc.vector.tensor_tensor(
                    out=oh[:],
                    in0=degrel[:, b * T_DEG:(b + 1) * T_DEG, None]
                        .to_broadcast([P, T_DEG, P]),
                    in1=iota[:, None, :].to_broadcast([P, T_DEG, P]),
                    op=mybir.AluOpType.is_equal,
                )
                pd = auxpool.tile([P, 1], F32, tag="aux", space="PSUM")
                for t in range(T_DEG):
                    nc.tensor.matmul(out=pd[:], lhsT=oh[:, t, :],
                                     rhs=ones[:, :],
                                     start=(t == 0), stop=(t == T_DEG - 1))
                nc.vector.tensor_copy(deg[:, b:b + 1], pd[:])

            # dis = rsqrt(max(deg,1)) * (deg != 0)
            dis = spool.tile([P, NBLK], F32)
            mask = spool.tile([P, NBLK], F32)
            nc.vector.tensor_scalar(out=mask[:], in0=deg[:], scalar1=0.0,
                                    scalar2=None,
                                    op0=mybir.AluOpType.not_equal)
            nc.vector.tensor_scalar(out=dis[:], in0=deg[:], scalar1=1.0,
                                    scalar2=None, op0=mybir.AluOpType.max)
            nc.scalar.activation(dis[:], dis[:],
                                 mybir.ActivationFunctionType.Sqrt)
            nc.vector.reciprocal(dis[:], dis[:])
            nc.vector.tensor_mul(dis[:], dis[:], mask[:])
            negdis = spool.tile([P, NBLK], F32)
            nc.vector.tensor_scalar(out=negdis[:], in0=dis[:], scalar1=-1.0,
                                    scalar2=None, op0=mybir.AluOpType.mult)
            nd2 = spool.tile([P, NBLK], F32)
            nc.vector.tensor_scalar(out=nd2[:], in0=dis[:], scalar1=-2.0,
                                    scalar2=None, op0=mybir.AluOpType.mult)

            out_acc = spool.tile([P, NBLK, DO], F32)

            def accum_out(src, kk, first):
                for b in range(NBLK):
                    pt = auxpool.tile([DP, P], F32, tag="aux", space="PSUM")
                    nc.tensor.transpose(pt[:], src[:, b, :], ident[:])
                    tT = wpool.tile([DP, P], F32, tag="tT")
                    nc.scalar.activation(tT[:], pt[:], COPY)
                    po = auxpool.tile([P, DO], F32, tag="aux", space="PSUM")
                    nc.tensor.matmul(out=po[:], lhsT=tT[:D, :],
                                     rhs=wmat[:, kk * DO:(kk + 1) * DO],
                                     start=True, stop=True)
                    if first:
                        nc.vector.tensor_copy(out_acc[:, b, :], po[:])
                    else:
                        nc.vector.tensor_add(out_acc[:, b, :],
                                             out_acc[:, b, :], po[:])

            accum_out(xs, 0, True)

            # ---- h0 = dis * x -> bounce0 -> AG -> table0 ------------------
            h0 = hpool.tile([P, NBLK, DP], F32, tag="hk")
            for b in range(NBLK):
                nc.scalar.activation(h0[:, b, :], xs[:, b, :], COPY,
                                     scale=dis[:, b:b + 1])
            nc.sync.dma_start(
                bounce[0][:, :].rearrange("(p b) f -> p b f", p=P), h0[:])
            nc.gpsimd.collective_compute(
                "AllGather", mybir.AluOpType.bypass, replica_groups=rg,
                ins=[bounce[0][:, :].opt()], outs=[table[0][:, :].opt()],
            )

            # ---- Chebyshev steps ------------------------------------------
            T = [xs, None, None, None]
            for k in range(1, K):
                tnew = tpool.tile([P, NBLK, DP], F32, tag="tn")
                if k < K - 1:
                    hknew = hpool.tile([P, NBLK, DP], F32, tag="hk")
                else:
                    hknew = None
                tab = table[k - 1]
                for b in range(NBLK):
                    gath = wpool.tile([P, T_ALL, DP], F32, tag="gath")
                    ib = b * T_ALL * 8
                    nc.gpsimd.dma_gather(
                        out_ap=gath[:, :T_LO, :],
                        in_ap=tab[:, :],
                        idxs_ap=gidx[:, ib:ib + T_LO * 8],
                        num_idxs=T_LO * P, num_idxs_reg=T_LO * P,
                        elem_size=DP, single_packet=False,
                    )
                    if T_HI:
                        nc.gpsimd.dma_gather(
                            out_ap=gath[:, T_LO:, :],
                            in_ap=tab[cfg.SPLIT:, :],
                            idxs_ap=gidx[:, ib + T_LO * 8:ib + T_ALL * 8],
                            num_idxs=T_HI * P, num_idxs_reg=T_HI * P,
                            elem_size=DP, single_packet=False,
                        )
                    hi = wpool.tile([P, T_ALL, DP], BF16, tag="hi")
                    nc.scalar.activation(hi[:], gath[:], COPY)
                    lo = wpool.tile([P, T_ALL, DP], BF16, tag="lo")
                    nc.vector.scalar_tensor_tensor(
                        out=lo[:], in0=gath[:], scalar=1.0, in1=hi[:],
                        op0=mybir.AluOpType.mult,
                        op1=mybir.AluOpType.subtract,
                    )
                    oh = ohpool.tile([P, T_ALL, P], BF16, tag="oh")
                    nc.vector.tensor_tensor(
                        out=oh[:],
                        in0=colrel[:, b * T_ALL:(b + 1) * T_ALL, None]
                            .to_broadcast([P, T_ALL, P]),
                        in1=iota[:, None, :].to_broadcast([P, T_ALL, P]),
                        op=mybir.AluOpType.is_equal,
                    )
                    ps = pspool.tile([P, DP], F32, tag="ps", space="PSUM")
                    for t in range(T_ALL):
                        nc.tensor.matmul(out=ps[:], lhsT=oh[:, t, :],
                                         rhs=hi[:, t, :],
                                         start=(t == 0), stop=False)
                        nc.tensor.matmul(out=ps[:], lhsT=oh[:, t, :],
                                         rhs=lo[:, t, :],
                                         start=False, stop=(t == T_ALL - 1))
                    if k == 1:
                        nc.scalar.activation(tnew[:, b, :], ps[:], COPY,
                                             scale=negdis[:, b:b + 1])
                    else:
                        nc.vector.scalar_tensor_tensor(
                            out=tnew[:, b, :], in0=ps[:],
                            scalar=nd2[:, b:b + 1], in1=T[k - 2][:, b, :],
                            op0=mybir.AluOpType.mult,
                            op1=mybir.AluOpType.subtract,
                        )
                    if hknew is not None:
                        nc.scalar.activation(hknew[:, b, :], tnew[:, b, :],
                                             COPY, scale=dis[:, b:b + 1])
                T[k] = tnew
                accum_out(tnew, k, False)
                if hknew is not None:
                    nc.sync.dma_start(
                        bounce[k][:, :].rearrange("(p b) f -> p b f", p=P),
                        hknew[:])
                    nc.gpsimd.collective_compute(
                        "AllGather", mybir.AluOpType.bypass,
                        replica_groups=rg,
                        ins=[bounce[k][:, :].opt()],
                        outs=[table[k][:, :].opt()],
                    )

            # ---- bias + store ---------------------------------------------
            nc.vector.tensor_add(
                out_acc[:], out_acc[:],
                bias_sb[:, None, :].to_broadcast([P, NBLK, DO]))
            nc.sync.dma_start(
                t_out[:, :].rearrange("p (b f) -> p b f", b=NBLK),
                out_acc[:])

    nc.compile()
    return nc


# ----------------------------------------------------------------------------
# driver
# ----------------------------------------------------------------------------

_programs = {}
last_exec_ns = None
trace = False


def run(x, edge_index, weight, bias, cfg):
    global last_exec_ns
    common, per_core, cfg = host_prep(x, edge_index, weight, bias, cfg)
    key = cfg.key()
    if key not in _programs:
        _programs[key] = build_program(cfg)
    nc = _programs[key]
    in_maps = []
    for c in range(cfg.NC):
        m = dict(common)
        m.update(per_core[c])
        in_maps.append(m)
    res = bass_utils.run_bass_kernel_spmd(
        nc, in_maps, core_ids=list(range(cfg.NC)), trace=trace)
    last_exec_ns = res.exec_time_ns

    N, NSH, NBLK, DO = cfg.N, cfg.NSH, cfg.NBLK, cfg.DO
    out = np.empty((N, DO), np.float32)
    for c in range(cfg.NC):
        arr = np.asarray(res.results[c]["out"]).reshape(P, NBLK, DO)
        r0 = c * NSH
        nrows = min(NSH, N - r0)
        rr = np.arange(nrows)
        out[r0:r0 + nrows] = arr[rr % P, rr // P, :]
    return out


def kernel(x, edge_index, weight, bias):
    cfg = Cfg(50000, 48, 48, 4, 8, 32768, None, None, None)
    return run(np.asarray(x), np.asarray(edge_index),
               np.asarray(weight), np.asarray(bias), cfg)


# revision 7
# speedup vs baseline: 1.3922x; 1.3922x over previous
"""ChebConv (K=4, sym norm, lambda_max=2) forward on 8 Trainium2 NeuronCores.

Problem (hardcoded): x [50000, 48] f32, edge_index [2, 1.6M] int64,
weight [4, 48, 48] f32, bias [48] f32 -> out [50000, 48] f32.

Strategy
--------
Math: with lambda_max=2 and sym normalization, L_hat's off-diagonal weight for
edge (j->i) is -dis[j]*dis[i] with dis = deg^-1/2 (deg = source out-counts,
self-loops removed). So prop(h) = -dis * segsum_col(dis[row] * h[row]): all
per-edge weights fold into two dense per-node scalings around a plain
gather + segment-sum; no per-edge multiply on device.

Distribution: core c owns target nodes [c*6250, (c+1)*6250). Edges are
partitioned by target core and sorted into 128-wide target blocks. Per
Chebyshev step: gather dis-scaled features (dma_gather, 256B f32 elems) from
an AllGathered DRAM table, segment-sum via one-hot matmuls on the
TensorEngine (race-free, PSUM-accumulated), apply the recurrence on the
local shard, AllGather the rescaled shard for the next step. Features ride
through the matmul as bf16 hi+lo splits (exact to ~1e-6 rel).

deg uses the same one-hot matmul against a ones vector over a
row-partitioned copy of the edge list.

dma_gather indices are int16, so each block's gather stream is split into
[0, 32768) and [32768, ..) windows (second call gathers from a row-offset
view of the table).

Node order inside a shard is "blocked": node n (r = n % 6250) maps to table
row NPAD*(n//6250) + NBLK*(r%128) + r//128, making every dense DMA
partition-contiguous.
"""

import sys

for _p in ("/opt/trn_rl_repo", "/root/.axon_site/_ro/trn_rl_repo"):
    if _p not in sys.path:
        sys.path.insert(0, _p)

import numpy as np
import ml_dtypes

import concourse.bacc as bacc
import concourse.mybir as mybir
import concourse.tile as tile
from concourse import bass_utils

F32 = mybir.dt.float32
BF16 = mybir.dt.bfloat16
I16 = mybir.dt.int16

P = 128
import os
N_QUEUES = int(os.environ.get("KM_QUEUES", "4"))
PAD_NEG = os.environ.get("KM_PADNEG", "1") == "1"


class Cfg:
    def __init__(self, n_nodes, d_in, d_out, k_cheb, n_cores, split,
                 t_lo, t_hi, t_deg):
        self.N = n_nodes
        self.D = d_in
        self.DO = d_out
        self.K = k_cheb
        self.NC = n_cores
        self.SPLIT = split              # int16 gather window size
        self.NSH = n_nodes // n_cores   # nodes per core
        self.NBLK = (self.NSH + P - 1) // P
        self.NPAD = self.NBLK * P
        self.NTAB = self.NC * self.NPAD
        self.DP = 64                    # padded feature width (256B elems)
        self.T_LO = t_lo
        self.T_HI = t_hi
        self.T_ALL = (t_lo + t_hi) if t_lo is not None else None
        self.T_DEG = t_deg

    def key(self):
        return (self.N, self.D, self.DO, self.K, self.NC, self.SPLIT,
                self.T_LO, self.T_HI, self.T_DEG)


# ----------------------------------------------------------------------------
# host-side data prep
# ----------------------------------------------------------------------------

def _wrap_idx16(vals):
    """[n] ints -> [128, n/16] int16 wrapped layout (idx i at
    (16g + i%16, i//16), replicated for the 8 gpsimd cores)."""
    n = len(vals)
    assert n % 16 == 0
    grid = np.asarray(vals, np.int64).reshape(n // 16, 16).T.astype(np.int16)
    return np.ascontiguousarray(np.tile(grid, (8, 1)))


def host_prep(x, edge_index, weight, bias, cfg, sim_safe=False):
    N, D, DP, NC, NSH, NBLK = cfg.N, cfg.D, cfg.DP, cfg.NC, cfg.NSH, cfg.NBLK

    rows = np.asarray(edge_index[0], np.int64)
    cols = np.asarray(edge_index[1], np.int64)
    keep = rows != cols
    rows, cols = rows[keep], cols[keep]

    # blocked table row of the source node
    rsh = rows % NSH
    gidx = cfg.NPAD * (rows // NSH) + NBLK * (rsh % P) + rsh // P
    lo_mask = gidx < cfg.SPLIT

    csh = cols % NSH
    core_c = cols // NSH
    colblk = csh // P
    colrel = csh % P
    core_r = rows // NSH
    rowblk = rsh // P
    rowrel = rsh % P

    # ---- order edges + measure tile counts ------------------------------
    t_lo = t_hi = t_deg = 0
    per_core_sorted = []
    for c in range(NC):
        sel = np.nonzero(core_c == c)[0]
        order = np.lexsort((~lo_mask[sel], colblk[sel]))
        sel = sel[order]
        bb = colblk[sel]
        ll = lo_mask[sel]
        dsel = np.nonzero(core_r == c)[0]
        per_core_sorted.append((sel, bb, ll, dsel))
        for b in range(NBLK):
            inb = bb == b
            nlo = int(np.count_nonzero(inb & ll))
            nhi = int(np.count_nonzero(inb)) - nlo
            t_lo = max(t_lo, -(-nlo // P))
            t_hi = max(t_hi, -(-nhi // P))
        bc = np.bincount(rowblk[dsel], minlength=NBLK)
        t_deg = max(t_deg, int(-(-int(bc.max()) // P)))

    if cfg.T_LO is None:
        cfg = Cfg(cfg.N, cfg.D, cfg.DO, cfg.K, cfg.NC, cfg.SPLIT,
                  t_lo, t_hi, t_deg)
    else:
        assert (t_lo <= cfg.T_LO and t_hi <= cfg.T_HI
                and t_deg <= cfg.T_DEG), (t_lo, t_hi, t_deg)

    T_LO, T_HI, T_ALL, T_DEG = cfg.T_LO, cfg.T_HI, cfg.T_ALL, cfg.T_DEG

    # ---- per-core arrays -------------------------------------------------
    per_core = []
    for c in range(NC):
        sel, bb, ll, dsel = per_core_sorted[c]
        padv = 0 if (sim_safe or not PAD_NEG) else -1
        g_arr = np.full((NBLK, T_ALL * P), padv, np.int64)
        c_arr = np.full((NBLK, T_ALL * P), -1.0, np.float32)   # pad colrel -1
        for b in range(NBLK):
            inb = np.nonzero(bb == b)[0]
            nlo = int(np.count_nonzero(ll[inb]))
            nb = len(inb)
            lo_ids = sel[inb[:nlo]]
            hi_ids = sel[inb[nlo:]]
            g_arr[b, :nlo] = gidx[lo_ids]
            c_arr[b, :nlo] = colrel[lo_ids]
            g_arr[b, T_LO * P:T_LO * P + nb - nlo] = gidx[hi_ids] - cfg.SPLIT
            c_arr[b, T_LO * P:T_LO * P + nb - nlo] = colrel[hi_ids]
        idx_sb = _wrap_idx16(g_arr.reshape(-1))
        colrel_sb = np.ascontiguousarray(
            c_arr.reshape(NBLK * T_ALL, P).T).astype(ml_dtypes.bfloat16)

        d_arr = np.full((NBLK, T_DEG * P), -1.0, np.float32)
        dblk = rowblk[dsel]
        drel = rowrel[dsel]
        for b in range(NBLK):
            inb = np.nonzero(dblk == b)[0]
            d_arr[b, :len(inb)] = drel[inb]
        degrel_sb = np.ascontiguousarray(
            d_arr.reshape(NBLK * T_DEG, P).T).astype(ml_dtypes.bfloat16)

        xs = np.zeros((P, NBLK, DP), np.float32)
        r0 = c * NSH
        nrows = min(NSH, N - r0)
        xi = np.asarray(x[r0:r0 + nrows], np.float32)
        rr = np.arange(nrows)
        xs[rr % P, rr // P, :D] = xi
        x_sb = np.ascontiguousarray(xs.reshape(P, NBLK * DP))

        per_core.append({
            "gidx": idx_sb, "colrel": colrel_sb, "degrel": degrel_sb,
            "x_shard": x_sb,
        })

    W = np.asarray(weight, np.float32)
    wmat = np.ascontiguousarray(
        W.transpose(1, 0, 2).reshape(D, cfg.K * cfg.DO))
    bias_sb = np.ascontiguousarray(
        np.broadcast_to(np.asarray(bias, np.float32), (P, cfg.DO)))
    iota_sb = np.ascontiguousarray(
        np.broadcast_to(np.arange(P, dtype=np.float32), (P, P))
    ).astype(ml_dtypes.bfloat16)
    ident_sb = np.eye(P, dtype=np.float32)
    ones_sb = np.ones((P, 1), ml_dtypes.bfloat16)

    common = {"wmat": wmat, "bias": bias_sb, "iota": iota_sb,
              "ident": ident_sb, "ones": ones_sb}
    return common, per_core, cfg


# ----------------------------------------------------------------------------
# bass program
# ----------------------------------------------------------------------------

def build_program(cfg):
    NBLK, DP, D, DO, K = cfg.NBLK, cfg.DP, cfg.D, cfg.DO, cfg.K
    T_LO, T_HI, T_ALL, T_DEG = cfg.T_LO, cfg.T_HI, cfg.T_ALL, cfg.T_DEG
    NC, NPAD, NTAB = cfg.NC, cfg.NPAD, cfg.NTAB
    COPY = mybir.ActivationFunctionType.Copy

    nc = bacc.Bacc("TRN2", target_bir_lowering=False, debug=False,
                   num_devices=NC, num_swdge_queues=N_QUEUES)

    t_x = nc.dram_tensor("x_shard", [P, NBLK * DP], F32, kind="ExternalInput")
    t_gidx = nc.dram_tensor("gidx", [P, NBLK * T_ALL * 8], I16,
                            kind="ExternalInput")
    t_colrel = nc.dram_tensor("colrel", [P, NBLK * T_ALL], BF16,
                              kind="ExternalInput")
    t_degrel = nc.dram_tensor("degrel", [P, NBLK * T_DEG], BF16,
                              kind="ExternalInput")
    t_wmat = nc.dram_tensor("wmat", [D, K * DO], F32, kind="ExternalInput")
    t_bias = nc.dram_tensor("bias", [P, DO], F32, kind="ExternalInput")
    t_iota = nc.dram_tensor("iota", [P, P], BF16, kind="ExternalInput")
    t_ident = nc.dram_tensor("ident", [P, P], F32, kind="ExternalInput")
    t_ones = nc.dram_tensor("ones", [P, 1], BF16, kind="ExternalInput")
    t_out = nc.dram_tensor("out", [P, NBLK * DO], F32, kind="ExternalOutput")

    bounce = [nc.dram_tensor(f"bounce{k}", [NPAD, DP], F32)
              for k in range(K - 1)]
    table = [nc.dram_tensor(f"table{k}", [NTAB, DP], F32, addr_space="Shared")
             for k in range(K - 1)]
    rg = [list(range(NC))]

    with tile.TileContext(nc) as tc:
        with (
            tc.tile_pool(name="const", bufs=1) as cpool,
            tc.tile_pool(name="state", bufs=1) as spool,
            tc.tile_pool(name="tpool", bufs=3) as tpool,
            tc.tile_pool(name="hpool", bufs=2) as hpool,
            tc.tile_pool(name="work", bufs=2) as wpool,
            tc.tile_pool(name="ohp", bufs=2) as ohpool,
            tc.tile_pool(name="ps", bufs=3, space="PSUM") as pspool,
            tc.tile_pool(name="aux", bufs=4, space="PSUM") as auxpool,
        ):
            iota = cpool.tile([P, P], BF16)
            nc.sync.dma_start(iota[:], t_iota[:, :])
            ident = cpool.tile([P, P], F32)
            nc.sync.dma_start(ident[:], t_ident[:, :])
            ones = cpool.tile([P, 1], BF16)
            nc.sync.dma_start(ones[:], t_ones[:, :])
            bias_sb = cpool.tile([P, DO], F32)
            nc.sync.dma_start(bias_sb[:], t_bias[:, :])
            wmat = cpool.tile([D, K * DO], F32)
            nc.sync.dma_start(wmat[:], t_wmat[:, :])
            gidx = cpool.tile([P, NBLK * T_ALL * 8], I16)
            nc.sync.dma_start(gidx[:], t_gidx[:, :])
            colrel = cpool.tile([P, NBLK * T_ALL], BF16)
            nc.sync.dma_start(colrel[:], t_colrel[:, :])
            degrel = cpool.tile([P, NBLK * T_DEG], BF16)
            nc.sync.dma_start(degrel[:], t_degrel[:, :])
            xs = tpool.tile([P, NBLK, DP], F32, tag="tn")
            nc.sync.dma_start(xs[:], t_x[:, :].rearrange("p (b f) -> p b f",
                                                         b=NBLK))

            # ---- deg pass -------------------------------------------------
            deg = spool.tile([P, NBLK], F32)
            for b in range(NBLK):
                oh = ohpool.tile([P, T_DEG, P], BF16, tag="oh")
                nc.vector.tensor_tensor(
                    out=oh[:],
                    in0=degrel[:, b * T_DEG:(b + 1) * T_DEG, None]
                        .to_broadcast([P, T_DEG, P]),
                    in1=iota[:, None, :].to_broadcast([P, T_DEG, P]),
                    op=mybir.AluOpType.is_equal,
                )
                pd = auxpool.tile([P, 1], F32, tag="aux", space="PSUM")
                for t in range(T_DEG):
                    nc.tensor.matmul(out=pd[:], lhsT=oh[:, t, :],
                                     rhs=ones[:, :],
                                     start=(t == 0), stop=(t == T_DEG - 1))
                nc.vector.tensor_copy(deg[:, b:b + 1], pd[:])

            # dis = rsqrt(max(deg,1)) * (deg != 0)
            dis = spool.tile([P, NBLK], F32)
            mask = spool.tile([P, NBLK], F32)
            nc.vector.tensor_scalar(out=mask[:], in0=deg[:], scalar1=0.0,
                                    scalar2=None,
                                    op0=mybir.AluOpType.not_equal)
            nc.vector.tensor_scalar(out=dis[:], in0=deg[:], scalar1=1.0,
                                    scalar2=None, op0=mybir.AluOpType.max)
            nc.scalar.activation(dis[:], dis[:],
                                 mybir.ActivationFunctionType.Sqrt)
            nc.vector.reciprocal(dis[:], dis[:])
            nc.vector.tensor_mul(dis[:], dis[:], mask[:])
            negdis = spool.tile([P, NBLK], F32)
            nc.vector.tensor_scalar(out=negdis[:], in0=dis[:], scalar1=-1.0,
                                    scalar2=None, op0=mybir.AluOpType.mult)
            nd2 = spool.tile([P, NBLK], F32)
            nc.vector.tensor_scalar(out=nd2[:], in0=dis[:], scalar1=-2.0,
                                    scalar2=None, op0=mybir.AluOpType.mult)

            out_acc = spool.tile([P, NBLK, DO], F32)

            def accum_out(src, kk, first):
                for b in range(NBLK):
                    pt = auxpool.tile([DP, P], F32, tag="aux", space="PSUM")
                    nc.tensor.transpose(pt[:], src[:, b, :], ident[:])
                    tT = wpool.tile([DP, P], F32, tag="tT")
                    nc.scalar.activation(tT[:], pt[:], COPY)
                    po = auxpool.tile([P, DO], F32, tag="aux", space="PSUM")
                    nc.tensor.matmul(out=po[:], lhsT=tT[:D, :],
                                     rhs=wmat[:, kk * DO:(kk + 1) * DO],
                                     start=True, stop=True)
                    if first:
                        nc.vector.tensor_copy(out_acc[:, b, :], po[:])
                    else:
                        nc.vector.tensor_add(out_acc[:, b, :],
                                             out_acc[:, b, :], po[:])

            accum_out(xs, 0, True)

            # ---- h0 = dis * x -> bounce0 -> AG -> table0 ------------------
            h0 = hpool.tile([P, NBLK, DP], F32, tag="hk")
            for b in range(NBLK):
                nc.scalar.activation(h0[:, b, :], xs[:, b, :], COPY,
                                     scale=dis[:, b:b + 1])
            nc.sync.dma_start(
                bounce[0][:, :].rearrange("(p b) f -> p b f", p=P), h0[:])
            nc.gpsimd.collective_compute(
                "AllGather", mybir.AluOpType.bypass, replica_groups=rg,
                ins=[bounce[0][:, :].opt()], outs=[table[0][:, :].opt()],
            )

            # pre-zero the gather slots (trimmed -1 tails leave them unread
            # by DMA; stale NaNs would poison 0*NaN in PSUM)
            for _gi in range(2):
                gz = wpool.tile([P, T_ALL, DP], F32, tag="gath")
                nc.vector.memset(gz[:], 0.0)

            # ---- Chebyshev steps ------------------------------------------
            T = [xs, None, None, None]
            for k in range(1, K):
                tnew = tpool.tile([P, NBLK, DP], F32, tag="tn")
                if k < K - 1:
                    hknew = hpool.tile([P, NBLK, DP], F32, tag="hk")
                else:
                    hknew = None
                tab = table[k - 1]
                qctr = 0
                for b in range(NBLK):
                    gath = wpool.tile([P, T_ALL, DP], F32, tag="gath")
                    ib = b * T_ALL * 8
                    nc.gpsimd.dma_gather(
                        out_ap=gath[:, :T_LO, :],
                        in_ap=tab[:, :],
                        idxs_ap=gidx[:, ib:ib + T_LO * 8],
                        num_idxs=T_LO * P, num_idxs_reg=T_LO * P,
                        elem_size=DP, single_packet=False,
                        queue_num=(qctr := qctr + 1) % N_QUEUES,
                    )
                    if T_HI:
                        nc.gpsimd.dma_gather(
                            out_ap=gath[:, T_LO:, :],
                            in_ap=tab[cfg.SPLIT:, :],
                            idxs_ap=gidx[:, ib + T_LO * 8:ib + T_ALL * 8],
                            num_idxs=T_HI * P, num_idxs_reg=T_HI * P,
                            elem_size=DP, single_packet=False,
                            queue_num=(qctr := qctr + 1) % N_QUEUES,
                        )
                    hi = wpool.tile([P, T_ALL, DP], BF16, tag="hi")
                    nc.scalar.activation(hi[:], gath[:], COPY)
                    lo = wpool.tile([P, T_ALL, DP], BF16, tag="lo")
                    nc.vector.scalar_tensor_tensor(
                        out=lo[:], in0=gath[:], scalar=1.0, in1=hi[:],
                        op0=mybir.AluOpType.mult,
                        op1=mybir.AluOpType.subtract,
                    )
                    oh = ohpool.tile([P, T_ALL, P], BF16, tag="oh")
                    nc.vector.tensor_tensor(
                        out=oh[:],
                        in0=colrel[:, b * T_ALL:(b + 1) * T_ALL, None]
                            .to_broadcast([P, T_ALL, P]),
                        in1=iota[:, None, :].to_broadcast([P, T_ALL, P]),
                        op=mybir.AluOpType.is_equal,
                    )
                    ps = pspool.tile([P, DP], F32, tag="ps", space="PSUM")
                    for t in range(T_ALL):
                        nc.tensor.matmul(out=ps[:], lhsT=oh[:, t, :],
                                         rhs=hi[:, t, :],
                                         start=(t == 0), stop=False)
                        nc.tensor.matmul(out=ps[:], lhsT=oh[:, t, :],
                                         rhs=lo[:, t, :],
                                         start=False, stop=(t == T_ALL - 1))
                    if k == 1:
                        nc.scalar.activation(tnew[:, b, :], ps[:], COPY,
                                             scale=negdis[:, b:b + 1])
                    else:
                        nc.vector.scalar_tensor_tensor(
                            out=tnew[:, b, :], in0=ps[:],
                            scalar=nd2[:, b:b + 1], in1=T[k - 2][:, b, :],
                            op0=mybir.AluOpType.mult,
                            op1=mybir.AluOpType.subtract,
                        )
                    if hknew is not None:
                        nc.scalar.activation(hknew[:, b, :], tnew[:, b, :],
                                             COPY, scale=dis[:, b:b + 1])
                T[k] = tnew
                accum_out(tnew, k, False)
                if hknew is not None:
                    nc.sync.dma_start(
                        bounce[k][:, :].rearrange("(p b) f -> p b f", p=P),
                        hknew[:])
                    nc.gpsimd.collective_compute(
                        "AllGather", mybir.AluOpType.bypass,
                        replica_groups=rg,
                        ins=[bounce[k][:, :].opt()],
                        outs=[table[k][:, :].opt()],
                    )

            # ---- bias + store ---------------------------------------------
            nc.vector.tensor_add(
                out_acc[:], out_acc[:],
                bias_sb[:, None, :].to_broadcast([P, NBLK, DO]))
            nc.sync.dma_start(
                t_out[:, :].rearrange("p (b f) -> p b f", b=NBLK),
                out_acc[:])

    nc.compile()
    return nc


# ----------------------------------------------------------------------------
# driver
# ----------------------------------------------------------------------------

_programs = {}
last_exec_ns = None
trace = False


def run(x, edge_index, weight, bias, cfg):
    global last_exec_ns
    common, per_core, cfg = host_prep(x, edge_index, weight, bias, cfg)
    key = cfg.key()
    if key not in _programs:
        _programs[key] = build_program(cfg)
    nc = _programs[key]
    in_maps = []
    for c in range(cfg.NC):
        m = dict(common)
        m.update(per_core[c])
        in_maps.append(m)
    res = bass_utils.run_bass_kernel_spmd(
        nc, in_maps, core_ids=list(range(cfg.NC)), trace=trace)
    last_exec_ns = res.exec_time_ns

    N, NSH, NBLK, DO = cfg.N, cfg.NSH, cfg.NBLK, cfg.DO
    out = np.empty((N, DO), np.float32)
    for c in range(cfg.NC):
        arr = np.asarray(res.results[c]["out"]).reshape(P, NBLK, DO)
        r0 = c * NSH
        nrows = min(NSH, N - r0)
        rr = np.arange(nrows)
        out[r0:r0 + nrows] = arr[rr % P, rr // P, :]
    return out


def kernel(x, edge_index, weight, bias):
    cfg = Cfg(50000, 48, 48, 4, 8, 32768, None, None, None)
    return run(np.asarray(x), np.asarray(edge_index),
               np.asarray(weight), np.asarray(bias), cfg)
